# revision 34
# baseline (speedup 1.0000x reference)
"""Trainium2 Bass kernel for nn_BidirectionalTrustModel.

Computes, for each of N=65536 independent observation sequences:
  1. A sequential scan over T=64 steps updating a per-sequence trust
     interval [low, high] for 2 capability dims (sens, proc).
  2. trust = prod over dims of (sum_j d_j * m_j / sum_j m_j) where
     m is the 10-bin interval mask and d_j = (1+exp(beta*(req-s_j)))^(-zeta^2).
     (The reference's 10x10 outer-product normalization factorizes exactly.)

Sharding: pure data-parallel over N across 8 NeuronCores (8192 seqs/core).

Device algorithm (exact emulation of the reference scan, in a x20-scaled
integer domain held in fp16 -- every value is an integer in [-82, 120],
exactly representable, so all compares/min/max/adds match the reference's
fp32 branch semantics bit-for-bit):
  per step, with input planes A = succ ? 20*cap : 0, B = fail ? 20*cap : 40,
  FX = per-event fixup constant (see below):
    g1 = A > high ; Ap = A - 80*g1     (guard: succ above the interval)
    lo1 = max(low, Ap) ; lo2 = min(lo1, B)
    hs  = max(high, A)
    g2 = B < low  ; Bp = B + 80*g2     (guard: fail below the interval)
    high' = min(hs, Bp)
    eq = (lo2 == high') ; low' = eq ? V : lo2   (copy_predicated)
  where V = 20*cap + fixup-offset is the host-precomputed forced low value
  for a tie at this event (ties imply low'==high'==cap).
  This reproduces the reference's branch/fixup semantics exactly:
  succ and fail are mutually exclusive, high is never 0 (the high2==0 fixup
  branch is dead), and the A-neutral 0 can only lift low from -1 to 0,
  which is bisimilar (all caps are >= 1; identical masks and compares).
  FX encodes the reference's fp32 rounding of (cap - 0.1f) relative to the
  grid point below: -2 when fp32 lands exactly on it, -1.5 when it rounds
  above, -2.5 when below. Fixup values only ever compare against grid
  integers, so the half-offsets replicate every fp32 comparison outcome
  (validated exhaustively on 200k random sequences).

fp16 gives the DVE's 2x perf mode on the 7 tensor_tensor ops per step
(the 3 scalar_tensor_tensor ops run at 1x), and host-packed A/B planes
halve the DMA volume vs shipping caps + perf bits separately.
"""

import numpy as np

BINS = 10
T = 64
N_TOTAL = 65536
N_CORES = 8
P = 128                 # SBUF partitions
NC = N_TOTAL // N_CORES  # 8192 sequences per core
K = NC // P             # 64 free-dim columns per dim
W = 2 * K               # 128 state columns: col = dim*K + k
FW = T * W              # 8192 columns for the [P, T*W] A/B planes

_F32 = np.float32
STEPS = ((np.arange(BINS, dtype=np.float32) + _F32(0.5)) * _F32(0.1)).astype(np.float32)

# Per-cap fixup constants for the x20 domain: the reference computes
# low = cap - 0.1f in fp32, which lands exactly on / above / below the grid
# point two units down depending on the bin. -2 / -1.5 / -2.5 replicate
# every comparison against grid values.
_FIX_OFF = np.empty(BINS, np.float32)
for _k in range(BINS):
    _v = np.float32(STEPS[_k] - np.float32(0.1))
    _below = STEPS[_k - 1] if _k > 0 else np.float32(0.0)
    _FIX_OFF[_k] = -2.0 if _v == _below else (-1.5 if _v > _below else -2.5)

# t-chunk sizes for DMA pipelining (small first chunks let the scan start
# early; A/B need no device-side precompute, so chunks feed the DVE directly)
CHUNK_STEPS = [2, 6, 8, 8, 8, 8, 8, 8, 8]
assert sum(CHUNK_STEPS) == T

_NC_CACHE = {}


def _build_nc():
    import concourse.bass as bass
    import concourse.mybir as mybir
    import concourse.tile as tile
    from concourse.tile import ScopedClock

    dt = mybir.dt
    Alu = mybir.AluOpType
    Act = mybir.ActivationFunctionType

    class PatchedTileContext(tile.TileContext):
        """This walrus build only lowers ONE sem wait per SP Drain; split the
        tail drain's waits across extra drain instructions."""
        MAX_WAITS = 1

        def _drain_and_barrier(self, tick_clock, wait_clock):
            nc = self.nc
            drain_inst = nc.sync.drain()
            wait_clock.add_sem_waits(
                drain_inst.ins, ScopedClock({None: tick_clock.global_clock})
            )
            si = drain_inst.ins.sync_info
            kmax = self.MAX_WAITS
            if si is not None and si.on_wait and len(si.on_wait) > kmax:
                waits = list(si.on_wait)
                drain_inst.ins.sync_info = mybir.SyncInfo(
                    on_wait=waits[:kmax], on_update=list(si.on_update)
                )
                rest = waits[kmax:]
                for i in range(0, len(rest), kmax):
                    extra = nc.sync.drain()
                    extra.ins.sync_info = mybir.SyncInfo(
                        on_wait=rest[i : i + kmax], on_update=[]
                    )
            nc.all_engine_barrier()
            assert self.sems is not None
            popped = nc._tile_sem_poison_stack.pop()
            assert popped is self._sem_poison
            nc.clear_and_free_semaphores(list(self.sems.allocated().values()))
            nc.all_engine_barrier()

    def _split_sync_waits(nc):
        """This walrus build lowers at most ONE sync wait per instruction.
        Move extra waits onto same-engine NoOps inserted just before."""
        n_split = 0
        for f in nc.m.functions:
            for bb in f.blocks:
                il = bb.instructions
                new = []
                for ins in il:
                    si = ins.sync_info
                    if si is not None and si.on_wait and len(si.on_wait) > 1:
                        waits = list(si.on_wait)
                        for w in waits[:-1]:
                            nop = mybir.InstNoOp(name=f"I-wsplit-{nc.next_id()}")
                            nop.engine = ins.engine
                            nop.sync_info = mybir.SyncInfo(on_wait=[w], on_update=[])
                            nc.register_instruction(nop, overwrite=True)
                            new.append(nop)
                            n_split += 1
                        ins.sync_info = mybir.SyncInfo(
                            on_wait=[waits[-1]], on_update=list(si.on_update)
                        )
                    new.append(ins)
                il[:] = new
        return n_split

    nc = bass.Bass(target_bir_lowering=False, trn_type="TRN2")

    f32, f16 = dt.float32, dt.float16
    A_d = nc.declare_dram_parameter("Aplane", [P, FW], f16, isOutput=False)
    B_d = nc.declare_dram_parameter("Bplane", [P, FW], f16, isOutput=False)
    FX_d = nc.declare_dram_parameter("FXplane", [P, FW], f16, isOutput=False)
    bt_d = nc.declare_dram_parameter("bt", [P, K * BINS], f32, isOutput=False)
    bt20_d = nc.declare_dram_parameter("bt20", [P, K * BINS], f16, isOutput=False)
    reqb_s_d = nc.declare_dram_parameter("reqb_s", [P, K * BINS], f32, isOutput=False)
    reqb_p_d = nc.declare_dram_parameter("reqb_p", [P, K * BINS], f32, isOutput=False)
    bz_d = nc.declare_dram_parameter("bz", [P, 4], f32, isOutput=False)
    out_d = nc.declare_dram_parameter("trust", [P, K], f32, isOutput=True)

    with PatchedTileContext(nc) as tc:
        # Keep every pool open for the whole kernel: closing a pool lets
        # the stack allocator hand its SBUF range to the next pool, and
        # Tile then serializes the new pool's writers behind ALL of the
        # old pool's accessors (released-zone overlap hazard) -- which
        # destroys the DMA/scan pipeline.
        with tc.tile_pool(name="stage", bufs=3) as stage, \
             tc.tile_pool(name="state", bufs=1) as state_pool, \
             tc.tile_pool(name="scantmp", bufs=2) as stp, \
             tc.tile_pool(name="final", bufs=1) as fin:
            NCHUNK = len(CHUNK_STEPS)
            CHUNK_T0 = [sum(CHUNK_STEPS[:c]) for c in range(NCHUNK)]
            CHMAX = max(CHUNK_STEPS) * W
            A_chunks = []
            B_chunks = []
            FX_chunks = []
            for c in range(NCHUNK):
                CH = CHUNK_STEPS[c] * W
                At = stage.tile([P, CHMAX], f16, tag="Ach", name=f"Ach{c}")[:, :CH]
                Bt = stage.tile([P, CHMAX], f16, tag="Bch", name=f"Bch{c}")[:, :CH]
                Ft = stage.tile([P, CHMAX], f16, tag="Fch", name=f"Fch{c}")[:, :CH]
                sl = slice(CHUNK_T0[c] * W, CHUNK_T0[c] * W + CH)
                nc.sync.dma_start(At[:], A_d[:, sl])
                nc.sync.dma_start(Bt[:], B_d[:, sl])
                nc.sync.dma_start(Ft[:], FX_d[:, sl])
                A_chunks.append(At)
                B_chunks.append(Bt)
                FX_chunks.append(Ft)

            # ---- d-weights (independent of the scan; emitted first so
            # the ACT engine computes them while the scan runs) ----
            KB = K * BINS  # 640
            bt = fin.tile([P, KB], f32, tag="bt")
            bt20 = fin.tile([P, KB], f16, tag="bt20")
            reqb_s = fin.tile([P, KB], f32, tag="reqb_s")
            reqb_p = fin.tile([P, KB], f32, tag="reqb_p")
            bz = fin.tile([P, 4], f32, tag="bz")
            nc.sync.dma_start(bt[:], bt_d[:, :])
            nc.sync.dma_start(bt20[:], bt20_d[:, :])
            nc.sync.dma_start(reqb_s[:], reqb_s_d[:, :])
            nc.sync.dma_start(reqb_p[:], reqb_p_d[:, :])
            nc.sync.dma_start(bz[:], bz_d[:, :])

            nzz = fin.tile([P, 2], f32, tag="nzz")
            nc.gpsimd.tensor_tensor(nzz[:], bz[:, 2:4], bz[:, 2:4], Alu.mult)
            nc.gpsimd.tensor_scalar(nzz[:], nzz[:], -1.0, None, Alu.mult)

            d_tiles = []
            for dim, reqb in ((0, reqb_s), (1, reqb_p)):
                t1 = fin.tile([P, KB], f32, tag=f"t1_{dim}")
                sp = fin.tile([P, KB], f32, tag=f"sp_{dim}")
                dti = fin.tile([P, KB], f16, tag=f"d_{dim}")
                # d = exp(-zeta^2 * ln(1 + exp(beta * (req - s))))
                nc.gpsimd.tensor_tensor(t1[:], reqb[:], bt[:], Alu.subtract)
                nc.scalar.activation(sp[:], t1[:], Act.Exp,
                                     scale=bz[:, dim : dim + 1])
                nc.gpsimd.tensor_scalar(t1[:], sp[:], 1.0, None, Alu.add)
                nc.scalar.activation(sp[:], t1[:], Act.Ln)
                nc.scalar.activation(dti[:], sp[:], Act.Exp,
                                     scale=nzz[:, dim : dim + 1])
                d_tiles.append(dti)

            # ---- the scan (DVE + Pool, fp16 x20-integer domain) ----
            # State rotates through the stp pool (tags give double buffering):
            # step t writes fresh lo/hi tiles read by step t+1.
            lo_t = state_pool.tile([P, W], f16, tag="low")
            hi_t = state_pool.tile([P, W], f16, tag="high")
            nc.vector.memset(lo_t[:], 0.0)
            nc.vector.memset(hi_t[:], 20.0)

            t_to_chunk = []
            for c in range(NCHUNK):
                t_to_chunk += [(c, i) for i in range(CHUNK_STEPS[c])]
            lo = lo_t[:]
            hi = hi_t[:]
            for t in range(T):
                tc_idx, tl_idx = t_to_chunk[t]
                A = A_chunks[tc_idx][:, tl_idx * W : (tl_idx + 1) * W]
                B = B_chunks[tc_idx][:, tl_idx * W : (tl_idx + 1) * W]
                V = FX_chunks[tc_idx][:, tl_idx * W : (tl_idx + 1) * W]
                g1 = stp.tile([P, W], f16, tag="g1")
                Ap = stp.tile([P, W], f16, tag="Ap")
                lo1 = stp.tile([P, W], f16, tag="lo1")
                lo2 = stp.tile([P, W], f16, tag="lo2")
                hs = stp.tile([P, W], f16, tag="hs")
                g2 = stp.tile([P, W], f16, tag="g2")
                Bp = stp.tile([P, W], f16, tag="Bp")
                eq = stp.tile([P, W], dt.uint16, tag="eq")
                hi2 = stp.tile([P, W], f16, tag="hi2")

                # All scan ops on DVE (Pool only supports arithmetic ALU ops
                # and is ~2.6x slower per op; it handles d-weights instead).
                # Ordering interleaves the succ/fail sub-chains so dependent
                # ops are not back-to-back.
                # succ-guard: Ap = A*[A <= high] (0 is the safe A-neutral);
                # fail-guard: Bp = B + 80*(B < low).
                nc.vector.tensor_tensor(g1[:], A, hi, Alu.is_le)
                nc.vector.tensor_tensor(g2[:], B, lo, Alu.is_lt)
                nc.vector.tensor_tensor(hs[:], hi, A, Alu.max)
                nc.vector.tensor_tensor(Ap[:], A, g1[:], Alu.mult)
                nc.vector.scalar_tensor_tensor(
                    Bp[:], g2[:], 80.0, B, Alu.mult, Alu.add)
                nc.vector.tensor_tensor(lo1[:], lo, Ap[:], Alu.max)
                nc.vector.tensor_tensor(hi2[:], hs[:], Bp[:], Alu.min)
                nc.vector.tensor_tensor(lo2[:], lo1[:], B, Alu.min)
                # tie fixup: low' = eq ? V : lo2 (V = forced value, from host)
                nc.vector.tensor_tensor(eq[:], lo2[:], hi2[:], Alu.is_equal)
                nc.vector.copy_predicated(lo2[:], eq[:], V)
                lo = lo2[:]
                hi = hi2[:]

            # ---- final phase (tail after the scan) ----
            # lo/hi stay fp16 in the x20 domain; compare against fp16 bt20.
            # Mask/d math in fp16 (values are 0/1 and (0,1] weights; well
            # within the fp16 envelope for the 2e-2 gate), reduces accumulate
            # to f32. dim0's chain on DVE, C-reduces + tail divide on Pool.
            bt3 = bt20[:].rearrange("p (k j) -> p k j", j=BINS)
            UC = []
            for dim in (0, 1):
                lowv = lo[:, dim * K : (dim + 1) * K]
                highv = hi[:, dim * K : (dim + 1) * K]
                lowb = lowv.unsqueeze(2).broadcast_to((P, K, BINS))
                highb = highv.unsqueeze(2).broadcast_to((P, K, BINS))
                m1 = fin.tile([P, KB], f16, tag=f"m1_{dim}")
                m = fin.tile([P, KB], f16, tag=f"m_{dim}")
                dm = fin.tile([P, KB], f16, tag=f"dm_{dim}")

                nc.vector.tensor_tensor(m1[:].rearrange("p (k j) -> p k j", j=BINS),
                                        bt3, lowb, Alu.is_ge)
                nc.vector.tensor_tensor(m[:].rearrange("p (k j) -> p k j", j=BINS),
                                        bt3, highb, Alu.is_le)
                nc.vector.tensor_tensor(m[:], m[:], m1[:], Alu.mult)
                nc.vector.tensor_tensor(dm[:], d_tiles[dim][:], m[:], Alu.mult)

                U = fin.tile([P, K], f32, tag=f"U_{dim}")
                C = fin.tile([P, K], f32, tag=f"C_{dim}")
                nc.vector.tensor_reduce(
                    U[:], dm[:].rearrange("p (k j) -> p k j", j=BINS),
                    mybir.AxisListType.X, Alu.add)
                nc.vector.tensor_reduce(
                    C[:], m[:].rearrange("p (k j) -> p k j", j=BINS),
                    mybir.AxisListType.X, Alu.add)
                UC.append((U, C))

            # trust = (U0*U1) / (C0*C1)
            uu = fin.tile([P, K], f32, tag="uu")
            cc = fin.tile([P, K], f32, tag="cc")
            rr = fin.tile([P, K], f32, tag="rr")
            tr = fin.tile([P, K], f32, tag="tr")
            nc.vector.tensor_tensor(uu[:], UC[0][0][:], UC[1][0][:], Alu.mult)
            nc.gpsimd.tensor_tensor(cc[:], UC[0][1][:], UC[1][1][:], Alu.mult)
            nc.vector.reciprocal(rr[:], cc[:])
            nc.vector.tensor_tensor(tr[:], uu[:], rr[:], Alu.mult)
            nc.sync.dma_start(out_d[:, :], tr[:])

    _split_sync_waits(nc)
    return nc


def _get_nc():
    if "nc" not in _NC_CACHE:
        _NC_CACHE["nc"] = _build_nc()
    return _NC_CACHE["nc"]


def _marshal_core(inputs, c):
    """Build the per-core input map (slicing/layout/packing, no model math)."""
    n0, n1 = c * NC, (c + 1) * NC

    # caps scaled x20 are odd integers 1..19: exact in fp16.
    obs_s = np.asarray(inputs["obs_task_sens_cap_seq"][:, n0:n1], dtype=np.float32) * np.float32(20.0)
    obs_p = np.asarray(inputs["obs_task_proc_cap_seq"][:, n0:n1], dtype=np.float32) * np.float32(20.0)
    perf = np.asarray(inputs["inptasksperf"][:, n0:n1, :])
    s1 = perf[:, :, 1] != 0   # success bit [T, NC]
    s0 = perf[:, :, 0] != 0   # fail bit

    # A = succ ? cap20 : 0 ; B = fail ? cap20 : 40, for both dims,
    # laid out [P, T*W] with col = t*W + dim*K + k, seq n = p*K + k.
    def lay(x):  # [T, 2, NC] -> [P, T*2*K]
        return np.ascontiguousarray(
            x.reshape(T, 2, P, K).transpose(2, 0, 1, 3).reshape(P, FW))

    zero = np.float32(0.0)
    forty = np.float32(40.0)
    A = np.stack([np.where(s1, obs_s, zero), np.where(s1, obs_p, zero)], axis=1)
    B = np.stack([np.where(s0, obs_s, forty), np.where(s0, obs_p, forty)], axis=1)
    bins_s = np.round(obs_s * 0.5 - 0.5).astype(np.int64)
    bins_p = np.round(obs_p * 0.5 - 0.5).astype(np.int64)
    # V = forced low value on a tie at this event: 20*cap + fixup offset
    V = np.stack([obs_s + _FIX_OFF[bins_s], obs_p + _FIX_OFF[bins_p]], axis=1)
    A = lay(A).astype(np.float16)
    B = lay(B).astype(np.float16)
    V = lay(V).astype(np.float16)

    def layreq(x):  # [NC] -> [P, K*BINS] broadcast each seq over 10 bins
        r = x.reshape(P, K, 1)
        return np.ascontiguousarray(np.broadcast_to(r, (P, K, BINS)).reshape(P, K * BINS))

    req_s = layreq(np.asarray(inputs["pred_task_sens_cap"][n0:n1, 0], dtype=np.float32))
    req_p = layreq(np.asarray(inputs["pred_task_proc_cap"][n0:n1, 0], dtype=np.float32))
    bt = np.ascontiguousarray(np.broadcast_to(np.tile(STEPS, K), (P, K * BINS))).astype(np.float32)
    st20 = (np.arange(BINS, dtype=np.float32) * 2 + 1).astype(np.float32)  # exact odd ints
    bt20 = np.ascontiguousarray(np.broadcast_to(np.tile(st20, K), (P, K * BINS))).astype(np.float16)
    betas = np.asarray(inputs["betas"], dtype=np.float32)
    zetas = np.asarray(inputs["zetas"], dtype=np.float32)
    bz = np.ascontiguousarray(
        np.broadcast_to(np.concatenate([betas, zetas]).astype(np.float32), (P, 4)))
    return {
        "Aplane": A, "Bplane": B, "FXplane": V,
        "bt": bt, "bt20": bt20, "reqb_s": req_s, "reqb_p": req_p, "bz": bz,
    }


def kernel(**inputs) -> np.ndarray:
    from concourse.bass_utils import run_bass_kernel_spmd

    nc = _get_nc()
    in_maps = [_marshal_core(inputs, c) for c in range(N_CORES)]
    res = run_bass_kernel_spmd(nc, in_maps, core_ids=list(range(N_CORES)))
    out = np.empty((N_TOTAL, 1), dtype=np.float32)
    for c in range(N_CORES):
        out[c * NC : (c + 1) * NC, 0] = res.results[c]["trust"].reshape(NC)
    return out


# ---------------------------------------------------------------------------
# numpy mirror of the device algorithm (for validation only)
def _numpy_mirror(inputs):
    obs_s = np.asarray(inputs["obs_task_sens_cap_seq"], dtype=np.float32) * 20.0
    obs_p = np.asarray(inputs["obs_task_proc_cap_seq"], dtype=np.float32) * 20.0
    perf = np.asarray(inputs["inptasksperf"])
    p0 = (perf[:, :, 0] != 0)
    p1 = (perf[:, :, 1] != 0)
    betas = np.asarray(inputs["betas"], dtype=np.float32)
    zetas = np.asarray(inputs["zetas"], dtype=np.float32)
    req = [np.asarray(inputs["pred_task_sens_cap"][:, 0], dtype=np.float32),
           np.asarray(inputs["pred_task_proc_cap"][:, 0], dtype=np.float32)]
    N = obs_s.shape[1]
    trust = np.ones(N, dtype=np.float32)
    h = np.float16
    for dim, obs in ((0, obs_s), (1, obs_p)):
        bins = np.round(obs * 0.5 - 0.5).astype(np.int64)
        FXD = _FIX_OFF[bins].astype(h)
        low = np.zeros(N, h)
        high = np.full(N, 20.0, h)
        for t in range(T):
            A = np.where(p1[t], obs[t], np.float32(0.0)).astype(h)
            B = np.where(p0[t], obs[t], np.float32(40.0)).astype(h)
            g1 = (A <= high).astype(h)
            hs = np.maximum(high, A)
            Ap = (A * g1).astype(h)
            lo1 = np.maximum(low, Ap)
            lo2 = np.minimum(lo1, B)
            g2 = (B < low).astype(h)
            Bp = (np.float16(80.0) * g2 + B).astype(h)
            high = np.minimum(hs, Bp)
            eqm = lo2 == high
            V = (obs[t] + FXD[t].astype(np.float32)).astype(h)
            low = np.where(eqm, V, lo2).astype(h)
        lo32 = low.astype(np.float32)
        hi32 = high.astype(np.float32)
        st20 = (np.arange(BINS, dtype=np.float32) * 2 + 1).astype(np.float32)
        m = ((st20[None, :] >= lo32[:, None]) & (st20[None, :] <= hi32[:, None]))
        z2 = np.float32(zetas[dim]) * np.float32(zetas[dim])
        p = np.float32(betas[dim]) * (req[dim][:, None] - STEPS[None, :])
        d = np.exp(-z2 * np.log1p(np.exp(p.astype(np.float64))))
        u = (d * m).sum(1) / m.sum(1)
        trust = trust * u.astype(np.float32)
    return trust[:, None]


# revision 36
# speedup vs baseline: 1.0061x; 1.0061x over previous
"""Trainium2 Bass kernel for nn_BidirectionalTrustModel.

Computes, for each of N=65536 independent observation sequences:
  1. A sequential scan over T=64 steps updating a per-sequence trust
     interval [low, high] for 2 capability dims (sens, proc).
  2. trust = prod over dims of (sum_j d_j * m_j / sum_j m_j) where
     m is the 10-bin interval mask and d_j = (1+exp(beta*(req-s_j)))^(-zeta^2).
     (The reference's 10x10 outer-product normalization factorizes exactly.)

Sharding: pure data-parallel over N across 8 NeuronCores (8192 seqs/core).

Device algorithm (exact emulation of the reference scan, in a x20-scaled
integer domain held in fp16 -- every value is an integer in [-82, 120],
exactly representable, so all compares/min/max/adds match the reference's
fp32 branch semantics bit-for-bit):
  per step, with input planes A = succ ? 20*cap : 0, B = fail ? 20*cap : 40,
  FX = per-event fixup constant (see below):
    g1 = A > high ; Ap = A - 80*g1     (guard: succ above the interval)
    lo1 = max(low, Ap) ; lo2 = min(lo1, B)
    hs  = max(high, A)
    g2 = B < low  ; Bp = B + 80*g2     (guard: fail below the interval)
    high' = min(hs, Bp)
    eq = (lo2 == high') ; low' = eq ? V : lo2   (copy_predicated)
  where V = 20*cap + fixup-offset is the host-precomputed forced low value
  for a tie at this event (ties imply low'==high'==cap).
  This reproduces the reference's branch/fixup semantics exactly:
  succ and fail are mutually exclusive, high is never 0 (the high2==0 fixup
  branch is dead), and the A-neutral 0 can only lift low from -1 to 0,
  which is bisimilar (all caps are >= 1; identical masks and compares).
  FX encodes the reference's fp32 rounding of (cap - 0.1f) relative to the
  grid point below: -2 when fp32 lands exactly on it, -1.5 when it rounds
  above, -2.5 when below. Fixup values only ever compare against grid
  integers, so the half-offsets replicate every fp32 comparison outcome
  (validated exhaustively on 200k random sequences).

fp16 gives the DVE's 2x perf mode on the 7 tensor_tensor ops per step
(the 3 scalar_tensor_tensor ops run at 1x), and host-packed A/B planes
halve the DMA volume vs shipping caps + perf bits separately.
"""

import numpy as np

BINS = 10
T = 64
N_TOTAL = 65536
N_CORES = 8
P = 128                 # SBUF partitions
NC = N_TOTAL // N_CORES  # 8192 sequences per core
K = NC // P             # 64 free-dim columns per dim
W = 2 * K               # 128 state columns: col = dim*K + k
FW = T * W              # 8192 columns for the [P, T*W] A/B planes

_F32 = np.float32
STEPS = ((np.arange(BINS, dtype=np.float32) + _F32(0.5)) * _F32(0.1)).astype(np.float32)

# Per-cap fixup constants for the x20 domain: the reference computes
# low = cap - 0.1f in fp32, which lands exactly on / above / below the grid
# point two units down depending on the bin. -2 / -1.5 / -2.5 replicate
# every comparison against grid values.
_FIX_OFF = np.empty(BINS, np.float32)
for _k in range(BINS):
    _v = np.float32(STEPS[_k] - np.float32(0.1))
    _below = STEPS[_k - 1] if _k > 0 else np.float32(0.0)
    _FIX_OFF[_k] = -2.0 if _v == _below else (-1.5 if _v > _below else -2.5)

# t-chunk sizes for DMA pipelining (small first chunks let the scan start
# early; A/B need no device-side precompute, so chunks feed the DVE directly)
CHUNK_STEPS = [2, 6, 8, 8, 8, 8, 8, 8, 8]
assert sum(CHUNK_STEPS) == T

_NC_CACHE = {}


def _build_nc():
    import concourse.bass as bass
    import concourse.mybir as mybir
    import concourse.tile as tile
    from concourse.tile import ScopedClock

    dt = mybir.dt
    Alu = mybir.AluOpType
    Act = mybir.ActivationFunctionType

    class PatchedTileContext(tile.TileContext):
        """This walrus build only lowers ONE sem wait per SP Drain; split the
        tail drain's waits across extra drain instructions."""
        MAX_WAITS = 1

        def _drain_and_barrier(self, tick_clock, wait_clock):
            nc = self.nc
            drain_inst = nc.sync.drain()
            wait_clock.add_sem_waits(
                drain_inst.ins, ScopedClock({None: tick_clock.global_clock})
            )
            si = drain_inst.ins.sync_info
            kmax = self.MAX_WAITS
            if si is not None and si.on_wait and len(si.on_wait) > kmax:
                waits = list(si.on_wait)
                drain_inst.ins.sync_info = mybir.SyncInfo(
                    on_wait=waits[:kmax], on_update=list(si.on_update)
                )
                rest = waits[kmax:]
                for i in range(0, len(rest), kmax):
                    extra = nc.sync.drain()
                    extra.ins.sync_info = mybir.SyncInfo(
                        on_wait=rest[i : i + kmax], on_update=[]
                    )
            nc.all_engine_barrier()
            assert self.sems is not None
            popped = nc._tile_sem_poison_stack.pop()
            assert popped is self._sem_poison
            nc.clear_and_free_semaphores(list(self.sems.allocated().values()))
            nc.all_engine_barrier()

    def _split_sync_waits(nc):
        """This walrus build lowers at most ONE sync wait per instruction.
        Move extra waits onto same-engine NoOps inserted just before."""
        n_split = 0
        for f in nc.m.functions:
            for bb in f.blocks:
                il = bb.instructions
                new = []
                for ins in il:
                    si = ins.sync_info
                    if si is not None and si.on_wait and len(si.on_wait) > 1:
                        waits = list(si.on_wait)
                        for w in waits[:-1]:
                            nop = mybir.InstNoOp(name=f"I-wsplit-{nc.next_id()}")
                            nop.engine = ins.engine
                            nop.sync_info = mybir.SyncInfo(on_wait=[w], on_update=[])
                            nc.register_instruction(nop, overwrite=True)
                            new.append(nop)
                            n_split += 1
                        ins.sync_info = mybir.SyncInfo(
                            on_wait=[waits[-1]], on_update=list(si.on_update)
                        )
                    new.append(ins)
                il[:] = new
        return n_split

    nc = bass.Bass(target_bir_lowering=False, trn_type="TRN2")

    f32, f16 = dt.float32, dt.float16
    A_d = nc.declare_dram_parameter("Aplane", [P, FW], f16, isOutput=False)
    B_d = nc.declare_dram_parameter("Bplane", [P, FW], f16, isOutput=False)
    FX_d = nc.declare_dram_parameter("FXplane", [P, FW], f16, isOutput=False)
    bt_d = nc.declare_dram_parameter("bt", [P, K * BINS], f32, isOutput=False)
    bt20_d = nc.declare_dram_parameter("bt20", [P, K * BINS], f16, isOutput=False)
    reqb_s_d = nc.declare_dram_parameter("reqb_s", [P, K * BINS], f32, isOutput=False)
    reqb_p_d = nc.declare_dram_parameter("reqb_p", [P, K * BINS], f32, isOutput=False)
    bz_d = nc.declare_dram_parameter("bz", [P, 4], f32, isOutput=False)
    out_d = nc.declare_dram_parameter("trust", [P, K], f32, isOutput=True)

    with PatchedTileContext(nc) as tc:
        # Keep every pool open for the whole kernel: closing a pool lets
        # the stack allocator hand its SBUF range to the next pool, and
        # Tile then serializes the new pool's writers behind ALL of the
        # old pool's accessors (released-zone overlap hazard) -- which
        # destroys the DMA/scan pipeline.
        with tc.tile_pool(name="stage", bufs=3) as stage, \
             tc.tile_pool(name="state", bufs=1) as state_pool, \
             tc.tile_pool(name="scantmp", bufs=2) as stp, \
             tc.tile_pool(name="final", bufs=1) as fin:
            NCHUNK = len(CHUNK_STEPS)
            CHUNK_T0 = [sum(CHUNK_STEPS[:c]) for c in range(NCHUNK)]
            CHMAX = max(CHUNK_STEPS) * W
            A_chunks = []
            B_chunks = []
            FX_chunks = []
            for c in range(NCHUNK):
                CH = CHUNK_STEPS[c] * W
                At = stage.tile([P, CHMAX], f16, tag="Ach", name=f"Ach{c}")[:, :CH]
                Bt = stage.tile([P, CHMAX], f16, tag="Bch", name=f"Bch{c}")[:, :CH]
                Ft = stage.tile([P, CHMAX], f16, tag="Fch", name=f"Fch{c}")[:, :CH]
                sl = slice(CHUNK_T0[c] * W, CHUNK_T0[c] * W + CH)
                nc.sync.dma_start(At[:], A_d[:, sl])
                nc.sync.dma_start(Bt[:], B_d[:, sl])
                nc.sync.dma_start(Ft[:], FX_d[:, sl])
                A_chunks.append(At)
                B_chunks.append(Bt)
                FX_chunks.append(Ft)

            # ---- d-weights (independent of the scan; emitted first so
            # the ACT engine computes them while the scan runs) ----
            KB = K * BINS  # 640
            bt = fin.tile([P, KB], f32, tag="bt")
            bt20 = fin.tile([P, KB], f16, tag="bt20")
            reqb_s = fin.tile([P, KB], f32, tag="reqb_s")
            reqb_p = fin.tile([P, KB], f32, tag="reqb_p")
            bz = fin.tile([P, 4], f32, tag="bz")
            nc.sync.dma_start(bt[:], bt_d[:, :])
            nc.sync.dma_start(bt20[:], bt20_d[:, :])
            nc.sync.dma_start(reqb_s[:], reqb_s_d[:, :])
            nc.sync.dma_start(reqb_p[:], reqb_p_d[:, :])
            nc.sync.dma_start(bz[:], bz_d[:, :])

            nzz = fin.tile([P, 2], f32, tag="nzz")
            nc.gpsimd.tensor_tensor(nzz[:], bz[:, 2:4], bz[:, 2:4], Alu.mult)
            nc.gpsimd.tensor_scalar(nzz[:], nzz[:], -1.0, None, Alu.mult)

            d_tiles = []
            for dim, reqb in ((0, reqb_s), (1, reqb_p)):
                t1 = fin.tile([P, KB], f32, tag=f"t1_{dim}")
                sp = fin.tile([P, KB], f32, tag=f"sp_{dim}")
                dti = fin.tile([P, KB], f16, tag=f"d_{dim}")
                # d = exp(-zeta^2 * ln(1 + exp(beta * (req - s))))
                nc.gpsimd.tensor_tensor(t1[:], reqb[:], bt[:], Alu.subtract)
                nc.scalar.activation(sp[:], t1[:], Act.Exp,
                                     scale=bz[:, dim : dim + 1])
                nc.gpsimd.tensor_scalar(t1[:], sp[:], 1.0, None, Alu.add)
                nc.scalar.activation(sp[:], t1[:], Act.Ln)
                nc.scalar.activation(dti[:], sp[:], Act.Exp,
                                     scale=nzz[:, dim : dim + 1])
                d_tiles.append(dti)

            # ---- the scan (DVE + Pool, fp16 x20-integer domain) ----
            # State rotates through the stp pool (tags give double buffering):
            # step t writes fresh lo/hi tiles read by step t+1.
            lo_t = state_pool.tile([P, W], f16, tag="low")
            hi_t = state_pool.tile([P, W], f16, tag="high")
            nc.vector.memset(lo_t[:], 0.0)
            nc.vector.memset(hi_t[:], 20.0)

            t_to_chunk = []
            for c in range(NCHUNK):
                t_to_chunk += [(c, i) for i in range(CHUNK_STEPS[c])]
            lo = lo_t[:]
            hi = hi_t[:]
            for t in range(T):
                tc_idx, tl_idx = t_to_chunk[t]
                A = A_chunks[tc_idx][:, tl_idx * W : (tl_idx + 1) * W]
                B = B_chunks[tc_idx][:, tl_idx * W : (tl_idx + 1) * W]
                V = FX_chunks[tc_idx][:, tl_idx * W : (tl_idx + 1) * W]
                g1 = stp.tile([P, W], f16, tag="g1")
                Ap = stp.tile([P, W], f16, tag="Ap")
                lo1 = stp.tile([P, W], f16, tag="lo1")
                lo2 = stp.tile([P, W], f16, tag="lo2")
                hs = stp.tile([P, W], f16, tag="hs")
                g2 = stp.tile([P, W], f16, tag="g2")
                Bp = stp.tile([P, W], f16, tag="Bp")
                eq = stp.tile([P, W], dt.uint16, tag="eq")
                hi2 = stp.tile([P, W], f16, tag="hi2")

                # All scan ops on DVE (Pool only supports arithmetic ALU ops
                # and is ~2.6x slower per op; it handles d-weights instead).
                # Ordering interleaves the succ/fail sub-chains so dependent
                # ops are not back-to-back.
                # succ-guard: Ap = A*[A <= high] (0 is the safe A-neutral);
                # fail-guard: Bp = B + 80*(B < low).
                nc.vector.tensor_tensor(g1[:], A, hi, Alu.is_le)
                nc.vector.tensor_tensor(g2[:], B, lo, Alu.is_lt)
                nc.vector.tensor_tensor(hs[:], hi, A, Alu.max)
                nc.gpsimd.tensor_tensor(Ap[:], A, g1[:], Alu.mult)
                nc.vector.scalar_tensor_tensor(
                    Bp[:], g2[:], 80.0, B, Alu.mult, Alu.add)
                nc.vector.tensor_tensor(lo1[:], lo, Ap[:], Alu.max)
                nc.vector.tensor_tensor(hi2[:], hs[:], Bp[:], Alu.min)
                nc.vector.tensor_tensor(lo2[:], lo1[:], B, Alu.min)
                # tie fixup: low' = eq ? V : lo2 (V = forced value, from host)
                nc.vector.tensor_tensor(eq[:], lo2[:], hi2[:], Alu.is_equal)
                nc.vector.copy_predicated(lo2[:], eq[:], V)
                lo = lo2[:]
                hi = hi2[:]

            # ---- final phase (tail after the scan) ----
            # lo/hi stay fp16 in the x20 domain; compare against fp16 bt20.
            # Mask/d math in fp16 (values are 0/1 and (0,1] weights; well
            # within the fp16 envelope for the 2e-2 gate), reduces accumulate
            # to f32. dim0's chain on DVE, C-reduces + tail divide on Pool.
            bt3 = bt20[:].rearrange("p (k j) -> p k j", j=BINS)
            UC = []
            for dim in (0, 1):
                lowv = lo[:, dim * K : (dim + 1) * K]
                highv = hi[:, dim * K : (dim + 1) * K]
                lowb = lowv.unsqueeze(2).broadcast_to((P, K, BINS))
                highb = highv.unsqueeze(2).broadcast_to((P, K, BINS))
                m1 = fin.tile([P, KB], f16, tag=f"m1_{dim}")
                m = fin.tile([P, KB], f16, tag=f"m_{dim}")
                dm = fin.tile([P, KB], f16, tag=f"dm_{dim}")

                nc.vector.tensor_tensor(m1[:].rearrange("p (k j) -> p k j", j=BINS),
                                        bt3, lowb, Alu.is_ge)
                nc.vector.tensor_tensor(m[:].rearrange("p (k j) -> p k j", j=BINS),
                                        bt3, highb, Alu.is_le)
                nc.vector.tensor_tensor(m[:], m[:], m1[:], Alu.mult)
                nc.vector.tensor_tensor(dm[:], d_tiles[dim][:], m[:], Alu.mult)

                U = fin.tile([P, K], f32, tag=f"U_{dim}")
                C = fin.tile([P, K], f32, tag=f"C_{dim}")
                nc.vector.tensor_reduce(
                    U[:], dm[:].rearrange("p (k j) -> p k j", j=BINS),
                    mybir.AxisListType.X, Alu.add)
                nc.vector.tensor_reduce(
                    C[:], m[:].rearrange("p (k j) -> p k j", j=BINS),
                    mybir.AxisListType.X, Alu.add)
                UC.append((U, C))

            # trust = (U0*U1) / (C0*C1)
            uu = fin.tile([P, K], f32, tag="uu")
            cc = fin.tile([P, K], f32, tag="cc")
            rr = fin.tile([P, K], f32, tag="rr")
            tr = fin.tile([P, K], f32, tag="tr")
            nc.vector.tensor_tensor(uu[:], UC[0][0][:], UC[1][0][:], Alu.mult)
            nc.gpsimd.tensor_tensor(cc[:], UC[0][1][:], UC[1][1][:], Alu.mult)
            nc.vector.reciprocal(rr[:], cc[:])
            nc.vector.tensor_tensor(tr[:], uu[:], rr[:], Alu.mult)
            nc.sync.dma_start(out_d[:, :], tr[:])

    _split_sync_waits(nc)
    return nc


def _get_nc():
    if "nc" not in _NC_CACHE:
        _NC_CACHE["nc"] = _build_nc()
    return _NC_CACHE["nc"]


def _marshal_core(inputs, c):
    """Build the per-core input map (slicing/layout/packing, no model math)."""
    n0, n1 = c * NC, (c + 1) * NC

    # caps scaled x20 are odd integers 1..19: exact in fp16.
    obs_s = np.asarray(inputs["obs_task_sens_cap_seq"][:, n0:n1], dtype=np.float32) * np.float32(20.0)
    obs_p = np.asarray(inputs["obs_task_proc_cap_seq"][:, n0:n1], dtype=np.float32) * np.float32(20.0)
    perf = np.asarray(inputs["inptasksperf"][:, n0:n1, :])
    s1 = perf[:, :, 1] != 0   # success bit [T, NC]
    s0 = perf[:, :, 0] != 0   # fail bit

    # A = succ ? cap20 : 0 ; B = fail ? cap20 : 40, for both dims,
    # laid out [P, T*W] with col = t*W + dim*K + k, seq n = p*K + k.
    def lay(x):  # [T, 2, NC] -> [P, T*2*K]
        return np.ascontiguousarray(
            x.reshape(T, 2, P, K).transpose(2, 0, 1, 3).reshape(P, FW))

    zero = np.float32(0.0)
    forty = np.float32(40.0)
    A = np.stack([np.where(s1, obs_s, zero), np.where(s1, obs_p, zero)], axis=1)
    B = np.stack([np.where(s0, obs_s, forty), np.where(s0, obs_p, forty)], axis=1)
    bins_s = np.round(obs_s * 0.5 - 0.5).astype(np.int64)
    bins_p = np.round(obs_p * 0.5 - 0.5).astype(np.int64)
    # V = forced low value on a tie at this event: 20*cap + fixup offset
    V = np.stack([obs_s + _FIX_OFF[bins_s], obs_p + _FIX_OFF[bins_p]], axis=1)
    A = lay(A).astype(np.float16)
    B = lay(B).astype(np.float16)
    V = lay(V).astype(np.float16)

    def layreq(x):  # [NC] -> [P, K*BINS] broadcast each seq over 10 bins
        r = x.reshape(P, K, 1)
        return np.ascontiguousarray(np.broadcast_to(r, (P, K, BINS)).reshape(P, K * BINS))

    req_s = layreq(np.asarray(inputs["pred_task_sens_cap"][n0:n1, 0], dtype=np.float32))
    req_p = layreq(np.asarray(inputs["pred_task_proc_cap"][n0:n1, 0], dtype=np.float32))
    bt = np.ascontiguousarray(np.broadcast_to(np.tile(STEPS, K), (P, K * BINS))).astype(np.float32)
    st20 = (np.arange(BINS, dtype=np.float32) * 2 + 1).astype(np.float32)  # exact odd ints
    bt20 = np.ascontiguousarray(np.broadcast_to(np.tile(st20, K), (P, K * BINS))).astype(np.float16)
    betas = np.asarray(inputs["betas"], dtype=np.float32)
    zetas = np.asarray(inputs["zetas"], dtype=np.float32)
    bz = np.ascontiguousarray(
        np.broadcast_to(np.concatenate([betas, zetas]).astype(np.float32), (P, 4)))
    return {
        "Aplane": A, "Bplane": B, "FXplane": V,
        "bt": bt, "bt20": bt20, "reqb_s": req_s, "reqb_p": req_p, "bz": bz,
    }


def kernel(**inputs) -> np.ndarray:
    from concourse.bass_utils import run_bass_kernel_spmd

    nc = _get_nc()
    in_maps = [_marshal_core(inputs, c) for c in range(N_CORES)]
    res = run_bass_kernel_spmd(nc, in_maps, core_ids=list(range(N_CORES)))
    out = np.empty((N_TOTAL, 1), dtype=np.float32)
    for c in range(N_CORES):
        out[c * NC : (c + 1) * NC, 0] = res.results[c]["trust"].reshape(NC)
    return out


# ---------------------------------------------------------------------------
# numpy mirror of the device algorithm (for validation only)
def _numpy_mirror(inputs):
    obs_s = np.asarray(inputs["obs_task_sens_cap_seq"], dtype=np.float32) * 20.0
    obs_p = np.asarray(inputs["obs_task_proc_cap_seq"], dtype=np.float32) * 20.0
    perf = np.asarray(inputs["inptasksperf"])
    p0 = (perf[:, :, 0] != 0)
    p1 = (perf[:, :, 1] != 0)
    betas = np.asarray(inputs["betas"], dtype=np.float32)
    zetas = np.asarray(inputs["zetas"], dtype=np.float32)
    req = [np.asarray(inputs["pred_task_sens_cap"][:, 0], dtype=np.float32),
           np.asarray(inputs["pred_task_proc_cap"][:, 0], dtype=np.float32)]
    N = obs_s.shape[1]
    trust = np.ones(N, dtype=np.float32)
    h = np.float16
    for dim, obs in ((0, obs_s), (1, obs_p)):
        bins = np.round(obs * 0.5 - 0.5).astype(np.int64)
        FXD = _FIX_OFF[bins].astype(h)
        low = np.zeros(N, h)
        high = np.full(N, 20.0, h)
        for t in range(T):
            A = np.where(p1[t], obs[t], np.float32(0.0)).astype(h)
            B = np.where(p0[t], obs[t], np.float32(40.0)).astype(h)
            g1 = (A <= high).astype(h)
            hs = np.maximum(high, A)
            Ap = (A * g1).astype(h)
            lo1 = np.maximum(low, Ap)
            lo2 = np.minimum(lo1, B)
            g2 = (B < low).astype(h)
            Bp = (np.float16(80.0) * g2 + B).astype(h)
            high = np.minimum(hs, Bp)
            eqm = lo2 == high
            V = (obs[t] + FXD[t].astype(np.float32)).astype(h)
            low = np.where(eqm, V, lo2).astype(h)
        lo32 = low.astype(np.float32)
        hi32 = high.astype(np.float32)
        st20 = (np.arange(BINS, dtype=np.float32) * 2 + 1).astype(np.float32)
        m = ((st20[None, :] >= lo32[:, None]) & (st20[None, :] <= hi32[:, None]))
        z2 = np.float32(zetas[dim]) * np.float32(zetas[dim])
        p = np.float32(betas[dim]) * (req[dim][:, None] - STEPS[None, :])
        d = np.exp(-z2 * np.log1p(np.exp(p.astype(np.float64))))
        u = (d * m).sum(1) / m.sum(1)
        trust = trust * u.astype(np.float32)
    return trust[:, None]
